# revision 53
# baseline (speedup 1.0000x reference)
"""Trainium2 Bass kernel for DepthWiseSeparableAttention.

Math notes (all exact identities, no approximations):
- The depthwise-conv "local bias" in the reference is constant along the
  softmax axis, so it cancels in softmax and is skipped entirely.
- Eval-mode BatchNorm, the LayerNorm affine (gamma/beta) and the attention
  scale fold into the qkv weight/bias on the host.
- K's effective bias adds a per-query constant to scores -> cancels in
  softmax -> dropped.  V's effective bias shifts attention output by a
  constant vector (softmax rows sum to 1) -> folded through proj_w into
  proj_b, which is added to the residual on-device (Pool engine).
- Softmax denominators come from a ones-column appended to V (the PV matmul
  then computes per-query colsums for free).

Measured-HW design points (microbenchmarked on the target cores):
- bf16 matmuls stream ~4 rows/cycle vs 1 for f32r -> all matmuls bf16.
- K=64 matmuls run ~0.75 rows/cycle regardless of dtype -> the per-head
  S = K^T Q (contraction DH=64) is zero-padded to K=128: the stationary
  holds one head's K-chunk in its 64 partition rows and zeros in the other
  64, the moving tensor is the full 128-row q block of the pair.
- exp([128,1024]) on ScalarE is 637ns -> attention is ScalarE-bound; the
  q/k/v projections are interleaved into the PE queue as filler so the PE
  never stalls waiting for exp, and probabilities are buffered so
  PV(head h) consumes while S/exp(head h+1) produces.
- HWDGE transfers do not block the issuing engine's compute -> x is split
  across the SP and ACT queues; normalization muls and the residual+bias
  adds run on the otherwise-idle Pool engine (which cannot touch PSUM, so
  all PSUM evictions stay on DVE).

Distribution: data-parallel over the batch dim - 8 batch elements, one per
NeuronCore, identical SPMD program, no collectives.
"""

import numpy as np

B, N, C = 8, 1024, 512
HEADS, DH = 8, 64
SCALE = DH ** -0.5
NT = N // 128   # 8 token chunks
CT = C // 128   # 4 channel chunks

P_BF16 = True

_CACHE = {}


def _build_program(p_bf16, loop_k=None, stop_after="full"):
    from contextlib import ExitStack

    import concourse.bacc as bacc
    import concourse.tile as tile
    from concourse import mybir
    from concourse.bass import ts

    f32 = mybir.dt.float32
    bf16 = mybir.dt.bfloat16
    fp16 = mybir.dt.float16
    Act = mybir.ActivationFunctionType
    Alu = mybir.AluOpType

    nc = bacc.Bacc(None, target_bir_lowering=False)

    x_d = nc.declare_dram_parameter("x", [N, C], f32, isOutput=False)
    wqk_d = nc.declare_dram_parameter("wqk", [C, 2 * C], bf16, isOutput=False)
    wv_d = nc.declare_dram_parameter("wv", [C, C], bf16, isOutput=False)
    pwt_d = nc.declare_dram_parameter("pwt", [C, C], bf16, isOutput=False)
    bq_d = nc.declare_dram_parameter("bq", [C], f32, isOutput=False)
    pb_d = nc.declare_dram_parameter("pb", [128, C], f32, isOutput=False)
    iden_d = nc.declare_dram_parameter("iden", [128, 128], bf16, isOutput=False)
    out_d = nc.declare_dram_parameter("out", [N, C], fp16, isOutput=True)

    with tile.TileContext(nc) as tc, ExitStack() as stk:
        const = stk.enter_context(tc.tile_pool(name="const", bufs=1))
        big = stk.enter_context(tc.tile_pool(name="big", bufs=1))

        wqk_sb = const.tile([128, CT, 2 * C], bf16)   # [p, cc, o]
        wv_sb = const.tile([128, CT, C], bf16)
        pwt_sb = const.tile([128, CT, C], bf16)
        bq_sb = const.tile([128, CT], f32)
        pb_sb = const.tile([128, C], f32)
        eps = const.tile([128, 1], f32)
        ones64 = const.tile([1, 64], fp16)
        iden = const.tile([128, 128], bf16)

        xnT = big.tile([128, CT, N], bf16)        # xn^T: [c_local, cc, tokens]
        qkT = big.tile([128, CT, N], bf16)        # q: [o_local, oc, tokens]
        kzT = big.tile([128, HEADS, N], bf16)     # k, zero-padded per head
        v_sb = big.tile([128, NT, HEADS, DH + 1], bf16)  # V natural + ones col
        ot = big.tile([128, CT, N], bf16)         # normalized O^T
        x_all = big.tile([128, NT, C], f32)       # x (natural layout)
        xr_all = big.tile([128, NT, C], f32)      # x + folded proj bias

        def phases():
            # ---- Phase 1: load + LayerNorm + XBAR transpose to xnT ---------
            with (
                tc.tile_pool(name="px", bufs=3) as px,
                tc.tile_pool(name="pstat", bufs=4) as pstat,
                tc.tile_pool(name="psA", bufs=2, space="PSUM") as psA,
            ):
                # x split across both HWDGE queues (SP + ACT) so LayerNorm
                # is not serialized behind a single 2MB stream
                x_r = x_d.rearrange("(t p) c -> p t c", p=128)
                for tcn in range(NT):
                    xq = nc.sync if tcn % 2 == 0 else nc.scalar
                    xq.dma_start(out=x_all[:, tcn, :], in_=x_r[:, tcn, :])
                wdma = nc.gpsimd
                wdma.dma_start(out=iden[:], in_=iden_d[:])
                wdma.dma_start(out=bq_sb[:],
                               in_=bq_d.rearrange("(cc p) -> p cc", p=128))
                wqk_r = wqk_d.rearrange("(cc p) o -> p cc o", p=128)
                wv_r = wv_d.rearrange("(cc p) o -> p cc o", p=128)
                pwt_r = pwt_d.rearrange("(cc p) o -> p cc o", p=128)
                for cc in range(CT):
                    wdma.dma_start(out=wqk_sb[:, cc, :], in_=wqk_r[:, cc, :])
                # zero-padding of the per-head K stationaries: heads 0/1 are
                # needed first (right after pair-0 projection) -> DVE; the
                # rest go on the Pool queue behind the wqk loads
                nc.vector.memset(kzT[64:128, 0, :], 0.0)
                nc.vector.memset(kzT[0:64, 1, :], 0.0)
                for h in range(2, HEADS):
                    half = slice(64, 128) if h % 2 == 0 else slice(0, 64)
                    nc.gpsimd.memset(kzT[half, h, :], 0.0)
                for cc in range(CT):
                    wdma.dma_start(out=wv_sb[:, cc, :], in_=wv_r[:, cc, :])
                for cc in range(CT):
                    wdma.dma_start(out=pwt_sb[:, cc, :], in_=pwt_r[:, cc, :])
                wdma.dma_start(out=pb_sb[:], in_=pb_d[:])
                nc.vector.memset(eps[:], 1e-6)
                nc.vector.memset(v_sb[:, :, :, DH:DH + 1], 1.0)
                nc.vector.memset(ones64[:], 1.0)

                for tcn in range(NT):
                    x_sb = x_all[:, tcn, :]
                    mean = pstat.tile([128, 1], f32, tag="mean")
                    nc.vector.tensor_reduce(out=mean[:], in_=x_sb[:],
                                            op=Alu.add, axis=mybir.AxisListType.X)
                    sq = px.tile([128, C], f32, tag="sq")
                    sumsq = pstat.tile([128, 1], f32, tag="sumsq")
                    nc.scalar.activation(out=sq[:], in_=x_sb[:], func=Act.Square,
                                         accum_out=sumsq[:])
                    nc.vector.tensor_scalar_mul(out=mean[:], in0=mean[:],
                                                scalar1=1.0 / C)
                    var = pstat.tile([128, 1], f32, tag="var")
                    nc.vector.tensor_tensor(out=var[:], in0=mean[:], in1=mean[:],
                                            op=Alu.mult)
                    nc.vector.tensor_scalar(out=var[:], in0=sumsq[:],
                                            scalar1=1.0 / C, scalar2=var[:],
                                            op0=Alu.mult, op1=Alu.subtract)
                    rstd = pstat.tile([128, 1], f32, tag="rstd")
                    nc.scalar.activation(out=rstd[:], in_=var[:], func=Act.Sqrt,
                                         bias=eps[:], scale=1.0)
                    nc.vector.reciprocal(out=rstd[:], in_=rstd[:])
                    # xn = x*rstd - mean*rstd on ScalarE, bf16 out for the XBAR
                    nb = pstat.tile([128, 1], f32, tag="nb")
                    nc.vector.tensor_scalar(out=nb[:], in0=mean[:],
                                            scalar1=rstd[:], scalar2=-1.0,
                                            op0=Alu.mult, op1=Alu.mult)
                    xn = px.tile([128, C], bf16, tag="xn")
                    with nc.allow_low_precision(reason="bf16 matmul inputs"):
                        nc.scalar.activation(out=xn[:], in_=x_sb[:],
                                             func=Act.Identity, bias=nb[:],
                                             scale=rstd[:])
                    pt = psA.tile([128, 512], bf16, tag="pt")
                    for cc in range(CT):
                        nc.tensor.transpose(pt[:, ts(cc, 128)],
                                            xn[:, ts(cc, 128)], iden[:])
                    with nc.allow_low_precision(reason="bf16 matmul inputs"):
                        nc.vector.tensor_copy(
                            out=xnT[:, :, ts(tcn, 128)],
                            in_=pt[:].rearrange("p (cc t) -> p cc t", cc=CT),
                        )

            if stop_after == "ln":
                return

            # ---- Fused qkv/v/attention stream ------------------------------
            with (
                tc.tile_pool(name="pp", bufs=3) as pp,
                tc.tile_pool(name="pr", bufs=2) as pr,
                tc.tile_pool(name="prd", bufs=4, space="DRAM") as prd,
                tc.tile_pool(name="psM", bufs=2, space="PSUM") as psM,
                tc.tile_pool(name="psS", bufs=2, space="PSUM") as psS,
                tc.tile_pool(name="psO", bufs=1, space="PSUM") as psO,
            ):
                def emit_qk_block(oc, nt):
                    # one [128 out-ch, 512 tokens] block of q or k projection
                    mm = psM.tile([128, 512], f32, tag="mm")
                    for cc in range(CT):
                        nc.tensor.matmul(
                            mm[:],
                            wqk_sb[:, cc, ts(oc, 128)],
                            xnT[:, cc, ts(nt, 512)],
                            start=(cc == 0), stop=(cc == CT - 1),
                        )
                    with nc.allow_low_precision(reason="bf16 matmul inputs"):
                        if oc < CT:  # q bias (k bias cancels in softmax)
                            nc.vector.tensor_scalar(
                                out=qkT[:, oc, ts(nt, 512)], in0=mm[:],
                                scalar1=bq_sb[:, oc:oc + 1], scalar2=None,
                                op0=Alu.add)
                        else:  # k: split heads into zero-padded stationaries
                            # (Pool engine; DVE is the busier one here)
                            p_idx = oc - CT
                            nc.vector.tensor_copy(
                                out=kzT[0:64, 2 * p_idx, ts(nt, 512)],
                                in_=mm[0:64, :])
                            nc.vector.tensor_copy(
                                out=kzT[64:128, 2 * p_idx + 1, ts(nt, 512)],
                                in_=mm[64:128, :])

                def emit_v(tcn):
                    mm = psM.tile([128, 512], f32, tag="mm")
                    for cc in range(CT):
                        nc.tensor.matmul(
                            mm[:],
                            xnT[:, cc, ts(tcn, 128)],
                            wv_sb[:, cc, :],
                            start=(cc == 0), stop=(cc == CT - 1),
                        )
                    with nc.allow_low_precision(reason="bf16 matmul inputs"):
                        nc.vector.tensor_copy(
                            out=v_sb[:, tcn, :, 0:DH],
                            in_=mm[:].rearrange("p (h d) -> p h d", h=HEADS),
                        )

                def emit_S_exp(h, kc, p_cur):
                    pr_ = h // 2
                    s = psS.tile([128, N], f32, tag="s")
                    for nt2 in range(2):
                        nc.tensor.matmul(
                            s[:, ts(nt2, 512)],
                            kzT[:, h, ts(kc, 128)],
                            qkT[:, pr_, ts(nt2, 512)],
                        )
                    nc.scalar.activation(out=p_cur[:, kc, :], in_=s[:],
                                         func=Act.Exp)

                def emit_norm(h, o_ps):
                    pr_ = h // 2
                    r16 = pr.tile([1, N], fp16, tag="r16", bufs=2)
                    o_raw = pr.tile([DH, N], f32, tag="oraw", bufs=2)
                    # drain o_ps in 512-column halves so the next head's
                    # PV(start) can claim the first PSUM half after ~1.2us
                    # instead of waiting for the full-width chain
                    for nt2 in range(2):
                        with nc.allow_low_precision(reason="normalization row"):
                            nc.vector.reciprocal(
                                out=r16[:, ts(nt2, 512)],
                                in_=o_ps[DH:DH + 1, ts(nt2, 512)])
                        nc.vector.tensor_copy(
                            out=o_raw[:, ts(nt2, 512)],
                            in_=o_ps[0:DH, ts(nt2, 512)])
                    rd = prd.tile([1, N], fp16, tag="rd")
                    nc.sync.dma_start(out=rd[:], in_=r16[:])
                    rb = pr.tile([64, N], fp16, tag="rb", bufs=2)
                    nc.sync.dma_start(out=rb[:],
                                      in_=rd[:].to_broadcast((64, N)))
                    with nc.allow_low_precision(reason="bf16 matmul inputs"):
                        if h % 2 == 0:
                            nc.gpsimd.tensor_mul(out=ot[0:64, pr_, :],
                                                 in0=o_raw[:], in1=rb[:])
                        else:
                            # odd head lands on partitions 64..127 of ot;
                            # engines cannot cross partitions, bounce via DMA
                            o_tmp = pr.tile([64, N], bf16, tag="otmp", bufs=2)
                            nc.gpsimd.tensor_mul(out=o_tmp[:],
                                                 in0=o_raw[:], in1=rb[:])
                            nc.sync.dma_start(out=ot[64:128, pr_, :],
                                              in_=o_tmp[:])

                def emit_norm_pe(h, o_ps):
                    # tail variant (even heads only): broadcast the
                    # reciprocal row across partitions with a K=1 matmul
                    # (ones64^T @ r) instead of a DRAM round-trip, so the
                    # output projection isn't blocked on DMA latency
                    pr_ = h // 2
                    r16 = pr.tile([1, N], fp16, tag="r16", bufs=2)
                    o_raw = pr.tile([DH, N], f32, tag="oraw", bufs=2)
                    # fully per-half so the output projection's first token
                    # chunks can start while the second half still normalizes
                    for nt2 in range(2):
                        with nc.allow_low_precision(reason="normalization"):
                            nc.vector.reciprocal(
                                out=r16[:, ts(nt2, 512)],
                                in_=o_ps[DH:DH + 1, ts(nt2, 512)])
                        nc.vector.tensor_copy(
                            out=o_raw[:, ts(nt2, 512)],
                            in_=o_ps[0:DH, ts(nt2, 512)])
                        bc = psM.tile([128, 512], f32, tag="mm")
                        nc.tensor.matmul(bc[0:64, :], ones64[:],
                                         r16[:, ts(nt2, 512)])
                        with nc.allow_low_precision(reason="bf16 inputs"):
                            nc.vector.tensor_mul(
                                out=ot[0:64, pr_, ts(nt2, 512)],
                                in0=o_raw[:, ts(nt2, 512)], in1=bc[0:64, :])

                if stop_after == "qkv":
                    for p_idx in range(4):
                        for oc in (p_idx, CT + p_idx):
                            for nt in range(2):
                                emit_qk_block(oc, nt)
                    for tcn in range(NT):
                        emit_v(tcn)
                    return

                # odd heads pay a DMA partition-shift in emit_norm; schedule
                # them first within each pair so the final head's (shorter)
                # normalization chain is what precedes the output projection
                order = [1, 0, 3, 2, 5, 4, 7, 6]

                # prologue: pair-0 projections, then first head's scores with
                # the v projection interleaved into the PE stream
                for oc in (0, CT):
                    for nt in range(2):
                        emit_qk_block(oc, nt)
                p_cur = pp.tile([128, NT, N], bf16, tag="p")
                for kc in range(NT):
                    emit_S_exp(order[0], kc, p_cur)
                    emit_v(kc)

                for hi in range(HEADS):
                    h = order[hi]
                    p_nxt = None
                    if hi < HEADS - 1:
                        p_nxt = pp.tile([128, NT, N], bf16, tag="p")
                    o_ps = psO.tile([DH + 1, N], f32, tag="o")
                    for kc in range(NT):
                        # S/exp of the next head first: at the head boundary
                        # PV below stalls on the previous head's PSUM drain,
                        # and the in-order PE queue would otherwise idle
                        if hi < HEADS - 1:
                            emit_S_exp(order[hi + 1], kc, p_nxt)
                        for nt2 in range(2):
                            nc.tensor.matmul(
                                o_ps[:, ts(nt2, 512)],
                                v_sb[:, kc, h, :],
                                p_cur[:, kc, ts(nt2, 512)],
                                start=(kc == 0), stop=(kc == NT - 1),
                            )
                        # during the first head of each pair, interleave the
                        # next pair's q/k projection blocks as PE filler
                        if hi % 2 == 0 and hi + 2 < HEADS and kc % 2 == 0:
                            p_idx = hi // 2 + 1
                            oc = p_idx if kc < 4 else CT + p_idx
                            emit_qk_block(oc, (kc // 2) % 2)
                    if hi == HEADS - 1:
                        emit_norm_pe(h, o_ps)
                    else:
                        emit_norm(h, o_ps)
                    p_cur = p_nxt

            if stop_after == "attn":
                return
            # ---- Phase 5: output projection + residual ---------------------
            with (
                tc.tile_pool(name="pout", bufs=3) as pout,
                tc.tile_pool(name="psY", bufs=4, space="PSUM") as psY,
            ):
                # residual-with-bias on the Pool engine, emitted here so the
                # adds don't queue ahead of the attention normalization work
                for tcn in range(NT):
                    nc.gpsimd.tensor_tensor(
                        out=xr_all[:, tcn, :], in0=x_all[:, tcn, :],
                        in1=pb_sb[:], op=Alu.add)
                for tcn in range(NT):
                    y_ps = psY.tile([128, 512], f32, tag="y")
                    for cc in range(CT):
                        nc.tensor.matmul(
                            y_ps[:],
                            ot[:, cc, ts(tcn, 128)],
                            pwt_sb[:, cc, :],
                            start=(cc == 0), stop=(cc == CT - 1),
                        )
                    y_sb = pout.tile([128, C], fp16, tag="y")
                    with nc.allow_low_precision(reason="fp16 output"):
                        nc.vector.tensor_add(out=y_sb[:], in0=y_ps[:],
                                             in1=xr_all[:, tcn, :])
                    oq = nc.sync if tcn % 2 == 0 else nc.scalar
                    oq.dma_start(out=out_d[ts(tcn, 128), :], in_=y_sb[:])

        if loop_k:
            with tc.For_i(0, loop_k, 1):
                phases()
        else:
            phases()

    nc.compile()
    return nc


def _prepare_host(inputs):
    import ml_dtypes
    f64 = np.float64
    bf16 = ml_dtypes.bfloat16
    x = np.asarray(inputs["x"], np.float32)
    qkv_w = np.asarray(inputs["qkv_w"], f64)
    qkv_b = np.asarray(inputs["qkv_b"], f64)
    g = np.asarray(inputs["ln_gamma"], f64)
    beta = np.asarray(inputs["ln_beta"], f64)
    s_bn = np.asarray(inputs["bn_gamma"], f64) / np.sqrt(
        np.asarray(inputs["bn_var"], f64) + 1e-5)
    bn_beta = np.asarray(inputs["bn_beta"], f64)
    bn_mean = np.asarray(inputs["bn_mean"], f64)
    proj_w = np.asarray(inputs["proj_w"], f64)
    proj_b = np.asarray(inputs["proj_b"], f64)

    w_eff = qkv_w * s_bn[:, None] * g[None, :]
    b_full = s_bn * (qkv_w @ beta + qkv_b - bn_mean) + bn_beta
    w_eff[0:C] *= SCALE
    b_full[0:C] *= SCALE

    wqk = np.ascontiguousarray(w_eff[0:2 * C].T).astype(bf16)     # [C, 2C]
    wv = np.ascontiguousarray(w_eff[2 * C:3 * C].T).astype(bf16)  # [C, C]
    pwt = np.ascontiguousarray(proj_w.T).astype(bf16)             # [C, C]
    bq = b_full[0:C].astype(np.float32)
    pb_eff = (proj_b + proj_w @ b_full[2 * C:3 * C]).astype(np.float32)
    pb128 = np.ascontiguousarray(
        np.broadcast_to(pb_eff[None, :], (128, C)), np.float32)
    iden = np.eye(128).astype(bf16)
    return x, wqk, wv, pwt, bq, pb128, iden


def _get_runner(nc):
    """Build (once) a jitted shard_map runner for the Bass program.

    run_bass_kernel_spmd re-traces and re-compiles the jitted wrapper on
    every call (~0.9s) and re-uploads every input (~40MB/s tunnel).  Here we
    cache the compiled callable + device buffers; per-call cost is then just
    the dispatch plus H2D for inputs whose bytes actually changed.
    """
    import jax
    from jax.sharding import Mesh, PartitionSpec, NamedSharding
    from jax.experimental.shard_map import shard_map
    from concourse import bass2jax as b2j
    from concourse import mybir

    b2j.install_neuronx_cc_hook()
    partition_name = (nc.partition_id_tensor.name
                      if nc.partition_id_tensor else None)
    in_names, out_names, out_avals, zero_outs = [], [], [], []
    for alloc in nc.m.functions[0].allocations:
        if not isinstance(alloc, mybir.MemoryLocationSet):
            continue
        name = alloc.memorylocations[0].name
        if alloc.kind == "ExternalInput":
            if name != partition_name:
                in_names.append(name)
        elif alloc.kind == "ExternalOutput":
            out_names.append(name)
            shape = tuple(alloc.tensor_shape)
            dtype = mybir.dt.np(alloc.dtype)
            out_avals.append(jax.core.ShapedArray(shape, dtype))
            zero_outs.append(np.zeros(shape, dtype))
    n_params = len(in_names)
    all_in_names = list(in_names) + list(out_names)
    if partition_name is not None:
        all_in_names.append(partition_name)

    def _body(*args):
        operands = list(args)
        if partition_name is not None:
            operands.append(b2j.partition_id_tensor())
        outs = b2j._bass_exec_p.bind(
            *operands,
            out_avals=tuple(out_avals),
            in_names=tuple(all_in_names),
            out_names=tuple(out_names),
            lowering_input_output_aliases=(),
            sim_require_finite=True,
            sim_require_nnan=True,
            nc=nc,
        )
        return tuple(outs)

    devices = jax.devices()[:B]
    mesh = Mesh(np.asarray(devices), ("core",))
    n_outs = len(out_names)
    fn = jax.jit(
        shard_map(_body, mesh=mesh,
                  in_specs=(PartitionSpec("core"),) * (n_params + n_outs),
                  out_specs=(PartitionSpec("core"),) * n_outs,
                  check_rep=False),
        keep_unused=True,
    )
    sharding = NamedSharding(mesh, PartitionSpec("core"))
    zeros_dev = [
        jax.device_put(np.zeros((B * z.shape[0], *z.shape[1:]), z.dtype),
                       sharding)
        for z in zero_outs
    ]
    return {
        "fn": fn, "sharding": sharding, "in_names": in_names,
        "out_names": out_names, "out_avals": out_avals,
        "zeros_dev": zeros_dev, "host_cache": {}, "dev_cache": {},
    }


def kernel(**inputs):
    import jax

    x, wqk, wv, pwt, bq, pb128, iden = _prepare_host(inputs)

    if "nc" not in _CACHE:
        _CACHE["nc"] = _build_program(P_BF16)
    nc = _CACHE["nc"]
    if "runner" not in _CACHE:
        _CACHE["runner"] = _get_runner(nc)
    rn = _CACHE["runner"]

    # Per-core values concatenated along axis 0 (shard_map hands each device
    # one slice).  Weights are identical across cores but the tunnel has no
    # multicast, so the only real saving is skipping re-uploads when bytes
    # are unchanged vs the cached copy.
    host_vals = {
        "x": np.ascontiguousarray(x.reshape(B * N, C)),
        "wqk": np.concatenate([wqk] * B, 0),
        "wv": np.concatenate([wv] * B, 0),
        "pwt": np.concatenate([pwt] * B, 0),
        "bq": np.concatenate([bq] * B, 0),
        "pb": np.concatenate([pb128] * B, 0),
        "iden": np.concatenate([iden] * B, 0),
    }
    dev_args = []
    for name in rn["in_names"]:
        hv = host_vals[name]
        cached = rn["host_cache"].get(name)
        if cached is None or not np.array_equal(cached, hv):
            rn["host_cache"][name] = hv
            rn["dev_cache"][name] = jax.device_put(hv, rn["sharding"])
        dev_args.append(rn["dev_cache"][name])
    out_arrs = rn["fn"](*dev_args, *rn["zeros_dev"])
    oi = rn["out_names"].index("out")
    out = np.asarray(out_arrs[oi]).reshape(B, N, C)
    return out.astype(np.float32)


# revision 57
# speedup vs baseline: 1.1136x; 1.1136x over previous
"""Trainium2 Bass kernel for DepthWiseSeparableAttention.

Math notes (all exact identities, no approximations):
- The depthwise-conv "local bias" in the reference is constant along the
  softmax axis, so it cancels in softmax and is skipped entirely.
- Eval-mode BatchNorm, the LayerNorm affine (gamma/beta) and the attention
  scale fold into the qkv weight/bias on the host.
- K's effective bias adds a per-query constant to scores -> cancels in
  softmax -> dropped.  V's effective bias shifts attention output by a
  constant vector (softmax rows sum to 1) -> folded through proj_w into
  proj_b, which is added to the residual on-device (Pool engine).
- Softmax denominators come from a ones-column appended to V (the PV matmul
  then computes per-query colsums for free).

Measured-HW design points (microbenchmarked on the target cores):
- bf16 matmuls stream ~4 rows/cycle vs 1 for f32r -> all matmuls bf16.
- K=64 matmuls run ~0.75 rows/cycle regardless of dtype -> the per-head
  S = K^T Q (contraction DH=64) is zero-padded to K=128: the stationary
  holds one head's K-chunk in its 64 partition rows and zeros in the other
  64, the moving tensor is the full 128-row q block of the pair.
- exp([128,1024]) on ScalarE is 637ns -> attention is ScalarE-bound; the
  q/k/v projections are interleaved into the PE queue as filler so the PE
  never stalls waiting for exp, and probabilities are buffered so
  PV(head h) consumes while S/exp(head h+1) produces.
- HWDGE transfers do not block the issuing engine's compute -> x is split
  across the SP and ACT queues; normalization muls and the residual+bias
  adds run on the otherwise-idle Pool engine (which cannot touch PSUM, so
  all PSUM evictions stay on DVE).

Distribution: data-parallel over the batch dim - 8 batch elements, one per
NeuronCore, identical SPMD program, no collectives.
"""

import numpy as np

B, N, C = 8, 1024, 512
HEADS, DH = 8, 64
SCALE = DH ** -0.5
NT = N // 128   # 8 token chunks
CT = C // 128   # 4 channel chunks

P_BF16 = True

_CACHE = {}


def _build_program(p_bf16, loop_k=None, stop_after="full"):
    from contextlib import ExitStack

    import concourse.bacc as bacc
    import concourse.tile as tile
    from concourse import mybir
    from concourse.bass import ts

    f32 = mybir.dt.float32
    bf16 = mybir.dt.bfloat16
    fp16 = mybir.dt.float16
    Act = mybir.ActivationFunctionType
    Alu = mybir.AluOpType

    nc = bacc.Bacc(None, target_bir_lowering=False)

    x_d = nc.declare_dram_parameter("x", [N, C], f32, isOutput=False)
    wqk_d = nc.declare_dram_parameter("wqk", [C, 2 * C], bf16, isOutput=False)
    wv_d = nc.declare_dram_parameter("wv", [C, C], bf16, isOutput=False)
    pwt_d = nc.declare_dram_parameter("pwt", [C, C], bf16, isOutput=False)
    bq_d = nc.declare_dram_parameter("bq", [C], f32, isOutput=False)
    pb_d = nc.declare_dram_parameter("pb", [128, C], f32, isOutput=False)
    iden_d = nc.declare_dram_parameter("iden", [128, 128], bf16, isOutput=False)
    out_d = nc.declare_dram_parameter("out", [N, C], fp16, isOutput=True)

    with tile.TileContext(nc) as tc, ExitStack() as stk:
        const = stk.enter_context(tc.tile_pool(name="const", bufs=1))
        big = stk.enter_context(tc.tile_pool(name="big", bufs=1))

        wqk_sb = const.tile([128, CT, 2 * C], bf16)   # [p, cc, o]
        wv_sb = const.tile([128, CT, C], bf16)
        pwt_sb = const.tile([128, CT, C], bf16)
        bq_sb = const.tile([128, CT], f32)
        pb_sb = const.tile([128, C], f32)
        eps = const.tile([128, 1], f32)
        ones64 = const.tile([1, 64], fp16)
        iden = const.tile([128, 128], bf16)

        xnT = big.tile([128, CT, N], bf16)        # xn^T: [c_local, cc, tokens]
        qkT = big.tile([128, CT, N], bf16)        # q: [o_local, oc, tokens]
        kzT = big.tile([128, HEADS, N], bf16)     # k, zero-padded per head
        v_sb = big.tile([128, NT, HEADS, DH + 1], bf16)  # V natural + ones col
        ot = big.tile([128, CT, N], bf16)         # normalized O^T
        x_all = big.tile([128, NT, C], f32)       # x (natural layout)
        xr_all = big.tile([128, NT, C], f32)      # x + folded proj bias

        def phases():
            # ---- Phase 1: load + LayerNorm + XBAR transpose to xnT ---------
            with (
                tc.tile_pool(name="px", bufs=3) as px,
                tc.tile_pool(name="pstat", bufs=4) as pstat,
                tc.tile_pool(name="psA", bufs=2, space="PSUM") as psA,
            ):
                # x split across both HWDGE queues (SP + ACT) so LayerNorm
                # is not serialized behind a single 2MB stream
                x_r = x_d.rearrange("(t p) c -> p t c", p=128)
                for tcn in range(NT):
                    xq = nc.sync if tcn % 2 == 0 else nc.scalar
                    xq.dma_start(out=x_all[:, tcn, :], in_=x_r[:, tcn, :])
                wdma = nc.gpsimd
                wdma.dma_start(out=iden[:], in_=iden_d[:])
                wdma.dma_start(out=bq_sb[:],
                               in_=bq_d.rearrange("(cc p) -> p cc", p=128))
                wqk_r = wqk_d.rearrange("(cc p) o -> p cc o", p=128)
                wv_r = wv_d.rearrange("(cc p) o -> p cc o", p=128)
                pwt_r = pwt_d.rearrange("(cc p) o -> p cc o", p=128)
                for cc in range(CT):
                    wdma.dma_start(out=wqk_sb[:, cc, :], in_=wqk_r[:, cc, :])
                # zero-padding of the per-head K stationaries: heads 0/1 are
                # needed first (right after pair-0 projection) -> DVE; the
                # rest go on the Pool queue behind the wqk loads
                nc.vector.memset(kzT[64:128, 0, :], 0.0)
                nc.vector.memset(kzT[0:64, 1, :], 0.0)
                for h in range(2, HEADS):
                    half = slice(64, 128) if h % 2 == 0 else slice(0, 64)
                    nc.gpsimd.memset(kzT[half, h, :], 0.0)
                for cc in range(CT):
                    wdma.dma_start(out=wv_sb[:, cc, :], in_=wv_r[:, cc, :])
                for cc in range(CT):
                    wdma.dma_start(out=pwt_sb[:, cc, :], in_=pwt_r[:, cc, :])
                wdma.dma_start(out=pb_sb[:], in_=pb_d[:])
                nc.vector.memset(eps[:], 1e-6)
                nc.vector.memset(v_sb[:, :, :, DH:DH + 1], 1.0)
                nc.vector.memset(ones64[:], 1.0)

                for tcn in range(NT):
                    x_sb = x_all[:, tcn, :]
                    mean = pstat.tile([128, 1], f32, tag="mean")
                    nc.vector.tensor_reduce(out=mean[:], in_=x_sb[:],
                                            op=Alu.add, axis=mybir.AxisListType.X)
                    sq = px.tile([128, C], f32, tag="sq")
                    sumsq = pstat.tile([128, 1], f32, tag="sumsq")
                    nc.scalar.activation(out=sq[:], in_=x_sb[:], func=Act.Square,
                                         accum_out=sumsq[:])
                    nc.vector.tensor_scalar_mul(out=mean[:], in0=mean[:],
                                                scalar1=1.0 / C)
                    var = pstat.tile([128, 1], f32, tag="var")
                    nc.vector.tensor_tensor(out=var[:], in0=mean[:], in1=mean[:],
                                            op=Alu.mult)
                    nc.vector.tensor_scalar(out=var[:], in0=sumsq[:],
                                            scalar1=1.0 / C, scalar2=var[:],
                                            op0=Alu.mult, op1=Alu.subtract)
                    rstd = pstat.tile([128, 1], f32, tag="rstd")
                    nc.scalar.activation(out=rstd[:], in_=var[:], func=Act.Sqrt,
                                         bias=eps[:], scale=1.0)
                    nc.vector.reciprocal(out=rstd[:], in_=rstd[:])
                    # xn = x*rstd - mean*rstd on ScalarE, bf16 out for the XBAR
                    nb = pstat.tile([128, 1], f32, tag="nb")
                    nc.vector.tensor_scalar(out=nb[:], in0=mean[:],
                                            scalar1=rstd[:], scalar2=-1.0,
                                            op0=Alu.mult, op1=Alu.mult)
                    xn = px.tile([128, C], bf16, tag="xn")
                    with nc.allow_low_precision(reason="bf16 matmul inputs"):
                        nc.scalar.activation(out=xn[:], in_=x_sb[:],
                                             func=Act.Identity, bias=nb[:],
                                             scale=rstd[:])
                    pt = psA.tile([128, 512], bf16, tag="pt")
                    for cc in range(CT):
                        nc.tensor.transpose(pt[:, ts(cc, 128)],
                                            xn[:, ts(cc, 128)], iden[:])
                    with nc.allow_low_precision(reason="bf16 matmul inputs"):
                        nc.vector.tensor_copy(
                            out=xnT[:, :, ts(tcn, 128)],
                            in_=pt[:].rearrange("p (cc t) -> p cc t", cc=CT),
                        )

            if stop_after == "ln":
                return

            # ---- Fused qkv/v/attention stream ------------------------------
            with (
                tc.tile_pool(name="pp", bufs=3) as pp,
                tc.tile_pool(name="pr", bufs=2) as pr,
                tc.tile_pool(name="prd", bufs=4, space="DRAM") as prd,
                tc.tile_pool(name="psM", bufs=2, space="PSUM") as psM,
                tc.tile_pool(name="psS", bufs=2, space="PSUM") as psS,
                tc.tile_pool(name="psO", bufs=1, space="PSUM") as psO,
            ):
                def emit_qk_block(oc, nt):
                    # one [128 out-ch, 512 tokens] block of q or k projection
                    mm = psM.tile([128, 512], f32, tag="mm")
                    for cc in range(CT):
                        nc.tensor.matmul(
                            mm[:],
                            wqk_sb[:, cc, ts(oc, 128)],
                            xnT[:, cc, ts(nt, 512)],
                            start=(cc == 0), stop=(cc == CT - 1),
                        )
                    with nc.allow_low_precision(reason="bf16 matmul inputs"):
                        if oc < CT:  # q bias (k bias cancels in softmax)
                            nc.vector.tensor_scalar(
                                out=qkT[:, oc, ts(nt, 512)], in0=mm[:],
                                scalar1=bq_sb[:, oc:oc + 1], scalar2=None,
                                op0=Alu.add)
                        else:  # k: split heads into zero-padded stationaries
                            # (Pool engine; DVE is the busier one here)
                            p_idx = oc - CT
                            nc.vector.tensor_copy(
                                out=kzT[0:64, 2 * p_idx, ts(nt, 512)],
                                in_=mm[0:64, :])
                            nc.vector.tensor_copy(
                                out=kzT[64:128, 2 * p_idx + 1, ts(nt, 512)],
                                in_=mm[64:128, :])

                def emit_v(tcn):
                    mm = psM.tile([128, 512], f32, tag="mm")
                    for cc in range(CT):
                        nc.tensor.matmul(
                            mm[:],
                            xnT[:, cc, ts(tcn, 128)],
                            wv_sb[:, cc, :],
                            start=(cc == 0), stop=(cc == CT - 1),
                        )
                    with nc.allow_low_precision(reason="bf16 matmul inputs"):
                        nc.vector.tensor_copy(
                            out=v_sb[:, tcn, :, 0:DH],
                            in_=mm[:].rearrange("p (h d) -> p h d", h=HEADS),
                        )

                def emit_S_exp(h, kc2, p_cur):
                    # two key chunks per score tile: one exp instruction
                    # (and one PE->ACT->PE sync round) covers 2048 columns,
                    # halving the semaphore traffic on the pacing engine
                    pr_ = h // 2
                    s = psS.tile([128, 2 * N], f32, tag="s", bufs=1)
                    for j in range(2):
                        kc = 2 * kc2 + j
                        for nt2 in range(2):
                            nc.tensor.matmul(
                                s[:, ts(2 * j + nt2, 512)],
                                kzT[:, h, ts(kc, 128)],
                                qkT[:, pr_, ts(nt2, 512)],
                            )
                    nc.scalar.activation(
                        out=p_cur[:, 2 * kc2:2 * kc2 + 2, :].rearrange(
                            "p a b -> p (a b)"),
                        in_=s[:], func=Act.Exp)

                def emit_norm(h, o_ps):
                    pr_ = h // 2
                    r16 = pr.tile([1, N], fp16, tag="r16", bufs=2)
                    o_raw = pr.tile([DH, N], f32, tag="oraw", bufs=2)
                    # drain o_ps in 512-column halves so the next head's
                    # PV(start) can claim the first PSUM half after ~1.2us
                    # instead of waiting for the full-width chain
                    for nt2 in range(2):
                        with nc.allow_low_precision(reason="normalization row"):
                            nc.vector.reciprocal(
                                out=r16[:, ts(nt2, 512)],
                                in_=o_ps[DH:DH + 1, ts(nt2, 512)])
                        nc.vector.tensor_copy(
                            out=o_raw[:, ts(nt2, 512)],
                            in_=o_ps[0:DH, ts(nt2, 512)])
                    rd = prd.tile([1, N], fp16, tag="rd")
                    nc.sync.dma_start(out=rd[:], in_=r16[:])
                    rb = pr.tile([64, N], fp16, tag="rb", bufs=2)
                    nc.sync.dma_start(out=rb[:],
                                      in_=rd[:].to_broadcast((64, N)))
                    with nc.allow_low_precision(reason="bf16 matmul inputs"):
                        if h % 2 == 0:
                            nc.gpsimd.tensor_mul(out=ot[0:64, pr_, :],
                                                 in0=o_raw[:], in1=rb[:])
                        else:
                            # odd head lands on partitions 64..127 of ot;
                            # engines cannot cross partitions, bounce via DMA
                            o_tmp = pr.tile([64, N], bf16, tag="otmp", bufs=2)
                            nc.gpsimd.tensor_mul(out=o_tmp[:],
                                                 in0=o_raw[:], in1=rb[:])
                            nc.sync.dma_start(out=ot[64:128, pr_, :],
                                              in_=o_tmp[:])

                def emit_norm_pe(h, o_ps):
                    # tail variant (even heads only): broadcast the
                    # reciprocal row across partitions with a K=1 matmul
                    # (ones64^T @ r) instead of a DRAM round-trip, so the
                    # output projection isn't blocked on DMA latency
                    pr_ = h // 2
                    r16 = pr.tile([1, N], fp16, tag="r16", bufs=2)
                    with nc.allow_low_precision(reason="normalization row"):
                        nc.vector.reciprocal(out=r16[:],
                                             in_=o_ps[DH:DH + 1, :])
                    o_raw = pr.tile([DH, N], f32, tag="oraw", bufs=2)
                    nc.vector.tensor_copy(out=o_raw[:], in_=o_ps[0:DH, :])
                    for nt2 in range(2):
                        bc = psM.tile([128, 512], f32, tag="mm")
                        nc.tensor.matmul(bc[0:64, :], ones64[:],
                                         r16[:, ts(nt2, 512)])
                        with nc.allow_low_precision(reason="bf16 inputs"):
                            nc.vector.tensor_mul(
                                out=ot[0:64, pr_, ts(nt2, 512)],
                                in0=o_raw[:, ts(nt2, 512)], in1=bc[0:64, :])

                if stop_after == "qkv":
                    for p_idx in range(4):
                        for oc in (p_idx, CT + p_idx):
                            for nt in range(2):
                                emit_qk_block(oc, nt)
                    for tcn in range(NT):
                        emit_v(tcn)
                    return

                # odd heads pay a DMA partition-shift in emit_norm; schedule
                # them first within each pair so the final head's (shorter)
                # normalization chain is what precedes the output projection
                order = [1, 0, 3, 2, 5, 4, 7, 6]

                # prologue: pair-0 projections, then first head's scores with
                # the v projection interleaved into the PE stream
                for oc in (0, CT):
                    for nt in range(2):
                        emit_qk_block(oc, nt)
                p_cur = pp.tile([128, NT, N], bf16, tag="p")
                for kc2 in range(NT // 2):
                    emit_S_exp(order[0], kc2, p_cur)
                    emit_v(2 * kc2)
                    emit_v(2 * kc2 + 1)

                for hi in range(HEADS):
                    h = order[hi]
                    p_nxt = None
                    if hi < HEADS - 1:
                        p_nxt = pp.tile([128, NT, N], bf16, tag="p")
                    o_ps = psO.tile([DH + 1, N], f32, tag="o")
                    for kc2 in range(NT // 2):
                        # S/exp of the next head first: at the head boundary
                        # PV below stalls on the previous head's PSUM drain,
                        # and the in-order PE queue would otherwise idle
                        if hi < HEADS - 1:
                            emit_S_exp(order[hi + 1], kc2, p_nxt)
                        for j in range(2):
                            kc = 2 * kc2 + j
                            for nt2 in range(2):
                                nc.tensor.matmul(
                                    o_ps[:, ts(nt2, 512)],
                                    v_sb[:, kc, h, :],
                                    p_cur[:, kc, ts(nt2, 512)],
                                    start=(kc == 0), stop=(kc == NT - 1),
                                )
                        # during the first head of each pair, interleave the
                        # next pair's q/k projection blocks as PE filler
                        if hi % 2 == 0 and hi + 2 < HEADS:
                            p_idx = hi // 2 + 1
                            oc = p_idx if kc2 < 2 else CT + p_idx
                            emit_qk_block(oc, kc2 % 2)
                    if hi == HEADS - 1:
                        emit_norm_pe(h, o_ps)
                    else:
                        emit_norm(h, o_ps)
                    p_cur = p_nxt

            if stop_after == "attn":
                return
            # ---- Phase 5: output projection + residual ---------------------
            with (
                tc.tile_pool(name="pout", bufs=3) as pout,
                tc.tile_pool(name="psY", bufs=4, space="PSUM") as psY,
            ):
                # residual-with-bias on the Pool engine, emitted here so the
                # adds don't queue ahead of the attention normalization work
                for tcn in range(NT):
                    nc.gpsimd.tensor_tensor(
                        out=xr_all[:, tcn, :], in0=x_all[:, tcn, :],
                        in1=pb_sb[:], op=Alu.add)
                for tcn in range(NT):
                    y_ps = psY.tile([128, 512], f32, tag="y")
                    for cc in range(CT):
                        nc.tensor.matmul(
                            y_ps[:],
                            ot[:, cc, ts(tcn, 128)],
                            pwt_sb[:, cc, :],
                            start=(cc == 0), stop=(cc == CT - 1),
                        )
                    y_sb = pout.tile([128, C], fp16, tag="y")
                    with nc.allow_low_precision(reason="fp16 output"):
                        nc.vector.tensor_add(out=y_sb[:], in0=y_ps[:],
                                             in1=xr_all[:, tcn, :])
                    oq = nc.sync if tcn % 2 == 0 else nc.scalar
                    oq.dma_start(out=out_d[ts(tcn, 128), :], in_=y_sb[:])

        if loop_k:
            with tc.For_i(0, loop_k, 1):
                phases()
        else:
            phases()

    nc.compile()
    return nc


def _prepare_host(inputs):
    import ml_dtypes
    f64 = np.float64
    bf16 = ml_dtypes.bfloat16
    x = np.asarray(inputs["x"], np.float32)
    qkv_w = np.asarray(inputs["qkv_w"], f64)
    qkv_b = np.asarray(inputs["qkv_b"], f64)
    g = np.asarray(inputs["ln_gamma"], f64)
    beta = np.asarray(inputs["ln_beta"], f64)
    s_bn = np.asarray(inputs["bn_gamma"], f64) / np.sqrt(
        np.asarray(inputs["bn_var"], f64) + 1e-5)
    bn_beta = np.asarray(inputs["bn_beta"], f64)
    bn_mean = np.asarray(inputs["bn_mean"], f64)
    proj_w = np.asarray(inputs["proj_w"], f64)
    proj_b = np.asarray(inputs["proj_b"], f64)

    w_eff = qkv_w * s_bn[:, None] * g[None, :]
    b_full = s_bn * (qkv_w @ beta + qkv_b - bn_mean) + bn_beta
    w_eff[0:C] *= SCALE
    b_full[0:C] *= SCALE

    wqk = np.ascontiguousarray(w_eff[0:2 * C].T).astype(bf16)     # [C, 2C]
    wv = np.ascontiguousarray(w_eff[2 * C:3 * C].T).astype(bf16)  # [C, C]
    pwt = np.ascontiguousarray(proj_w.T).astype(bf16)             # [C, C]
    bq = b_full[0:C].astype(np.float32)
    pb_eff = (proj_b + proj_w @ b_full[2 * C:3 * C]).astype(np.float32)
    pb128 = np.ascontiguousarray(
        np.broadcast_to(pb_eff[None, :], (128, C)), np.float32)
    iden = np.eye(128).astype(bf16)
    return x, wqk, wv, pwt, bq, pb128, iden


def _get_runner(nc):
    """Build (once) a jitted shard_map runner for the Bass program.

    run_bass_kernel_spmd re-traces and re-compiles the jitted wrapper on
    every call (~0.9s) and re-uploads every input (~40MB/s tunnel).  Here we
    cache the compiled callable + device buffers; per-call cost is then just
    the dispatch plus H2D for inputs whose bytes actually changed.
    """
    import jax
    from jax.sharding import Mesh, PartitionSpec, NamedSharding
    from jax.experimental.shard_map import shard_map
    from concourse import bass2jax as b2j
    from concourse import mybir

    b2j.install_neuronx_cc_hook()
    partition_name = (nc.partition_id_tensor.name
                      if nc.partition_id_tensor else None)
    in_names, out_names, out_avals, zero_outs = [], [], [], []
    for alloc in nc.m.functions[0].allocations:
        if not isinstance(alloc, mybir.MemoryLocationSet):
            continue
        name = alloc.memorylocations[0].name
        if alloc.kind == "ExternalInput":
            if name != partition_name:
                in_names.append(name)
        elif alloc.kind == "ExternalOutput":
            out_names.append(name)
            shape = tuple(alloc.tensor_shape)
            dtype = mybir.dt.np(alloc.dtype)
            out_avals.append(jax.core.ShapedArray(shape, dtype))
            zero_outs.append(np.zeros(shape, dtype))
    n_params = len(in_names)
    all_in_names = list(in_names) + list(out_names)
    if partition_name is not None:
        all_in_names.append(partition_name)

    def _body(*args):
        operands = list(args)
        if partition_name is not None:
            operands.append(b2j.partition_id_tensor())
        outs = b2j._bass_exec_p.bind(
            *operands,
            out_avals=tuple(out_avals),
            in_names=tuple(all_in_names),
            out_names=tuple(out_names),
            lowering_input_output_aliases=(),
            sim_require_finite=True,
            sim_require_nnan=True,
            nc=nc,
        )
        return tuple(outs)

    devices = jax.devices()[:B]
    mesh = Mesh(np.asarray(devices), ("core",))
    n_outs = len(out_names)
    fn = jax.jit(
        shard_map(_body, mesh=mesh,
                  in_specs=(PartitionSpec("core"),) * (n_params + n_outs),
                  out_specs=(PartitionSpec("core"),) * n_outs,
                  check_rep=False),
        keep_unused=True,
    )
    sharding = NamedSharding(mesh, PartitionSpec("core"))
    zeros_dev = [
        jax.device_put(np.zeros((B * z.shape[0], *z.shape[1:]), z.dtype),
                       sharding)
        for z in zero_outs
    ]
    return {
        "fn": fn, "sharding": sharding, "in_names": in_names,
        "out_names": out_names, "out_avals": out_avals,
        "zeros_dev": zeros_dev, "host_cache": {}, "dev_cache": {},
    }


def kernel(**inputs):
    import jax

    x, wqk, wv, pwt, bq, pb128, iden = _prepare_host(inputs)

    if "nc" not in _CACHE:
        _CACHE["nc"] = _build_program(P_BF16)
    nc = _CACHE["nc"]
    if "runner" not in _CACHE:
        _CACHE["runner"] = _get_runner(nc)
    rn = _CACHE["runner"]

    # Per-core values concatenated along axis 0 (shard_map hands each device
    # one slice).  Weights are identical across cores but the tunnel has no
    # multicast, so the only real saving is skipping re-uploads when bytes
    # are unchanged vs the cached copy.
    host_vals = {
        "x": np.ascontiguousarray(x.reshape(B * N, C)),
        "wqk": np.concatenate([wqk] * B, 0),
        "wv": np.concatenate([wv] * B, 0),
        "pwt": np.concatenate([pwt] * B, 0),
        "bq": np.concatenate([bq] * B, 0),
        "pb": np.concatenate([pb128] * B, 0),
        "iden": np.concatenate([iden] * B, 0),
    }
    dev_args = []
    for name in rn["in_names"]:
        hv = host_vals[name]
        cached = rn["host_cache"].get(name)
        if cached is None or not np.array_equal(cached, hv):
            rn["host_cache"][name] = hv
            rn["dev_cache"][name] = jax.device_put(hv, rn["sharding"])
        dev_args.append(rn["dev_cache"][name])
    out_arrs = rn["fn"](*dev_args, *rn["zeros_dev"])
    oi = rn["out_names"].index("out")
    out = np.asarray(out_arrs[oi]).reshape(B, N, C)
    return out.astype(np.float32)
